# revision 12
# baseline (speedup 1.0000x reference)
"""Trainium2 Bass kernel for the arm-sampling rollout problem.

Math: the reference's 2048-step scan x <- x - (A@x)*dt with
A = P diag(exp(D)) P^-1 has the closed form
    hidden[k] = P diag(lam_i^k) P^-1 x0,   lam_i = 1 - dt*exp(D_i)
so actions^T[ch, k] = tanh(sum_i G[ch,i] * c_i * lam_i^k + bm[ch]) with
G = Wm @ P and c = P^-1 x0. c is obtained on-device: unpivoted
Gauss-Jordan on [P^T | I] (same pivot sequence as P; well-conditioned
for this problem family) gives Q = P^-T, then c = matmul(lhsT=Q, rhs=x0).
The output is the memory-bound broadcast
    out[arm, j] = 150*eps[arm, j] + 15000*act_flat[j]
over a [5000, 4096] array, 625 arms per core across 8 cores.

Perf notes (v2, from HW perfetto traces):
- A dma_start whose SBUF side has exactly 128 partitions is split across
  all 16 SDMA engines (1 descriptor per 16KB partition line, ~330GB/s);
  a 113-partition transfer collapses onto ONE engine (~27GB/s). So every
  bulk tile here is exactly 128 rows; the last tile overlaps the
  previous one by 15 rows (identical bytes are double-written).
- fp32 PE matmuls cost 4 cycles/row; float32r with moving dim >= 256
  costs 1. The broadcast matmuls (ones (x) actions -> B) and the action
  matmuls run in f32r (~19-bit mantissa, plenty for the 2e-2 gate).
- lam^k is built by 11 multiplicative doublings instead of
  iota+cast+ln+exp (saves ~10us of gpsimd/scalar critical path).
- The elementwise main loop is split vector/gpsimd, outputs alternate
  scalar/sync HWDGE queues so input and output streams stay spread.
"""

import numpy as np

import concourse.bass as bass
import concourse.bacc as bacc
import concourse.mybir as mybir
import concourse.tile as tile
from concourse.bass_utils import run_bass_kernel_spmd

N_ARMS = 5000
N_STEPS = 2048
H = 10
F = 2 * N_STEPS  # 4096 flattened per-arm elements
N_CORES = 8
ARMS_PER_CORE = N_ARMS // N_CORES  # 625
OFFS = [0, 128, 256, 384, ARMS_PER_CORE - 128]  # all tiles exactly 128 rows
FP = mybir.dt.float32
FR = mybir.dt.float32r

_NC_CACHE: dict = {}


def build_nc():
    AFT = mybir.ActivationFunctionType
    ALU = mybir.AluOpType

    nc = bacc.Bacc(
        "TRN2",
        target_bir_lowering=False,
        debug=False,
        enable_asserts=True,
        num_devices=N_CORES,
    )

    eps_d = nc.dram_tensor("eps", [ARMS_PER_CORE, F], FP, kind="ExternalInput")
    tgt_d = nc.dram_tensor("target", [2], FP, kind="ExternalInput")
    D_d = nc.dram_tensor("D", [H], FP, kind="ExternalInput")
    P_d = nc.dram_tensor("P", [H, H], FP, kind="ExternalInput")
    W1_d = nc.dram_tensor("W1", [256, 2], FP, kind="ExternalInput")
    b1_d = nc.dram_tensor("b1", [256], FP, kind="ExternalInput")
    W2_d = nc.dram_tensor("W2", [H, 256], FP, kind="ExternalInput")
    b2_d = nc.dram_tensor("b2", [H], FP, kind="ExternalInput")
    Wm_d = nc.dram_tensor("Wm", [2, H], FP, kind="ExternalInput")
    bm_d = nc.dram_tensor("bm", [2], FP, kind="ExternalInput")
    out_d = nc.dram_tensor("out", [ARMS_PER_CORE, F], FP, kind="ExternalOutput")

    with tile.TileContext(nc) as tc:
        with (
            tc.tile_pool(name="sbc", bufs=1) as sbc,
            tc.tile_pool(name="sbgj", bufs=2) as sbgj,
            tc.tile_pool(name="sbeps", bufs=1) as sbeps,
            tc.tile_pool(name="psa", bufs=2, space=bass.MemorySpace.PSUM) as psa,
            tc.tile_pool(name="psbc", bufs=1, space=bass.MemorySpace.PSUM) as psbc,
            tc.tile_pool(name="psgc", bufs=1, space=bass.MemorySpace.PSUM) as psgc,
            tc.tile_pool(name="psact", bufs=2, space=bass.MemorySpace.PSUM) as psact,
            tc.tile_pool(name="psB", bufs=2, space=bass.MemorySpace.PSUM) as psB,
        ):
            # ---------- bulk in: 5 x [128, F] transfers on sync --------------
            eps_tiles = []
            for i, r in enumerate(OFFS):
                t = sbeps.tile([128, F], FP, tag="eps" + str(i))
                nc.sync.dma_start(t[:], eps_d.ap()[r : r + 128, :])
                eps_tiles.append((t, r))

            # ---------- small loads: plain 2D on scalar HWDGE ----------------
            ds = sbc.tile([H, 1], FP, tag="ds")
            nc.scalar.dma_start(ds[:], D_d.ap()[:, None])
            tgtr = sbc.tile([1, 2], FP, tag="tgtr")
            nc.scalar.dma_start(tgtr[:], tgt_d.ap()[None, :])
            b1n = sbc.tile([1, 256], FP, tag="b1n")
            nc.scalar.dma_start(b1n[:], b1_d.ap()[None, :])
            w2n = sbc.tile([H, 256], FP, tag="w2n")
            nc.scalar.dma_start(w2n[:], W2_d.ap())
            p_sb = sbc.tile([H, H], FP, tag="p_sb")
            nc.scalar.dma_start(p_sb[:], P_d.ap())
            b2s = sbc.tile([H, 1], FP, tag="b2s")
            nc.scalar.dma_start(b2s[:], b2_d.ap()[:, None])

            # ---------- transposing/odd loads on gpsimd SWDGE ----------------
            pT = sbc.tile([H, H], FP, tag="pT")
            nc.gpsimd.dma_start(pT[:], P_d.ap().rearrange("m k -> k m"))
            idi = sbc.tile([H, H], mybir.dt.int32, tag="idi")
            nc.gpsimd.iota(idi[:], pattern=[[-1, H]], base=0, channel_multiplier=1)
            wmT = sbc.tile([H, 2], FP, tag="wmT")
            nc.gpsimd.dma_start(wmT[:], Wm_d.ap().rearrange("m k -> k m"))
            w1n0 = sbc.tile([128, 2], FP, tag="w1n0")
            nc.gpsimd.dma_start(w1n0[:], W1_d.ap()[0:128, :])
            w1n1 = sbc.tile([128, 2], FP, tag="w1n1")
            nc.gpsimd.dma_start(w1n1[:], W1_d.ap()[128:256, :])
            bm0 = sbc.tile([1, 1], FP, tag="bm0")
            nc.gpsimd.dma_start(bm0[:], bm_d.ap()[0:1][:, None])
            bm1 = sbc.tile([1, 1], FP, tag="bm1")
            nc.gpsimd.dma_start(bm1[:], bm_d.ap()[1:2][:, None])

            # ---------- vector-early consts ----------------------------------
            ones = sbc.tile([1, 128], FP, tag="ones")
            nc.vector.memset(ones[:], 1.0)
            ones_r = sbc.tile([1, 128], FR, tag="ones_r")
            nc.vector.tensor_copy(ones_r[:], ones[:])
            # idm[p, j] = 1 if p == j
            idm = sbc.tile([H, H], FP, tag="idm")
            nc.vector.tensor_scalar(idm[:], idi[:], 0, None, ALU.is_equal)
            # oht[:, 10k:10k+10] = matrix with row k all-ones (lhsT that
            # broadcasts row k of the GJ tableau to every partition).
            oht = sbc.tile([H, H * H], FP, tag="oht")
            oht3 = oht[:].rearrange("p (k r) -> p k r", r=H)
            for r in range(H):
                nc.vector.tensor_copy(oht3[:, :, r : r + 1], idm[:, :, None])

            # ---------- lam = 1 - 0.01*exp(D); vc[i,k] = lam_i^k -------------
            es = sbc.tile([H, 1], FP, tag="es")
            nc.scalar.activation(es[:], ds[:], AFT.Exp)
            lam = sbc.tile([H, 1], FP, tag="lam")
            nc.vector.tensor_scalar(lam[:], es[:], -0.01, 1.0, ALU.mult, ALU.add)
            # multiplicative doubling: vc[:, w:2w] = vc[:, 0:w] * lam^w
            vc = sbc.tile([H, N_STEPS], FP, tag="vc")
            nc.vector.memset(vc[:, 0:1], 1.0)
            sqt = sbc.tile([H, 12], FP, tag="sqt")
            nc.vector.tensor_copy(sqt[:, 0:1], lam[:])
            w, j = 1, 0
            while w < N_STEPS:
                nc.vector.tensor_scalar_mul(
                    vc[:, w : 2 * w], vc[:, 0:w], sqt[:, j : j + 1]
                )
                if 2 * w < N_STEPS:
                    nc.vector.tensor_mul(
                        sqt[:, j + 1 : j + 2], sqt[:, j : j + 1], sqt[:, j : j + 1]
                    )
                w, j = 2 * w, j + 1
            # f32r copy for the PE (consumers of f32r matmuls must be rounded)
            vcr = sbc.tile([H, N_STEPS], FR, tag="vcr")
            nc.vector.tensor_copy(vcr[:], vc[:])

            # ---------- x0-path stationary transposes (PE, before GJ) --------
            tbp = psa.tile([128, 2], FP, tag="mm")
            nc.tensor.matmul(tbp[:], ones[:], tgtr[:])
            tb = sbc.tile([128, 2], FP, tag="tb")
            nc.vector.tensor_copy(tb[:], tbp[:])
            b1p0 = psa.tile([128, 1], FP, tag="mm")
            nc.tensor.matmul(
                b1p0[:], b1n[0:1, 0:128], ones[0:1, 0:1], is_transpose=True
            )
            b1a = sbc.tile([128, 1], FP, tag="b1a")
            nc.vector.tensor_copy(b1a[:], b1p0[:])
            b1p1 = psa.tile([128, 1], FP, tag="mm")
            nc.tensor.matmul(
                b1p1[:], b1n[0:1, 128:256], ones[0:1, 0:1], is_transpose=True
            )
            b1b = sbc.tile([128, 1], FP, tag="b1b")
            nc.vector.tensor_copy(b1b[:], b1p1[:])
            w2tp0 = psa.tile([128, H], FP, tag="mm")
            nc.tensor.matmul(w2tp0[:], w2n[:, 0:128], idm[:], is_transpose=True)
            w2t0 = sbc.tile([128, H], FP, tag="w2t0")
            nc.vector.tensor_copy(w2t0[:], w2tp0[:])
            w2tp1 = psa.tile([128, H], FP, tag="mm")
            nc.tensor.matmul(w2tp1[:], w2n[:, 128:256], idm[:], is_transpose=True)
            w2t1 = sbc.tile([128, H], FP, tag="w2t1")
            nc.vector.tensor_copy(w2t1[:], w2tp1[:])

            # ---------- Gauss-Jordan on [P^T | I] -> Q = P^-T ----------------
            aug = sbgj.tile([H, 2 * H], FP, tag="aug")
            nc.vector.tensor_copy(aug[:, 0:H], pT[:])
            nc.vector.tensor_copy(aug[:, H : 2 * H], idm[:])
            for k in range(H):
                fn = sbgj.tile([H, 1], FP, tag="fn")
                nc.vector.tensor_sub(fn[:], idm[:, k : k + 1], aug[:, k : k + 1])
                bc = psbc.tile([H, 2 * H], FP, tag="bc")
                nc.tensor.matmul(bc[:], oht[:, H * k : H * k + H], aug[:])
                piv = sbgj.tile([H, 1], FP, tag="piv")
                nc.vector.reciprocal(piv[:], bc[:, k : k + 1])
                fn2 = sbgj.tile([H, 1], FP, tag="fn2")
                nc.vector.tensor_mul(fn2[:], fn[:], piv[:])
                aug2 = sbgj.tile([H, 2 * H], FP, tag="aug")
                nc.vector.scalar_tensor_tensor(
                    aug2[:], bc[:], fn2[:], aug[:], ALU.mult, ALU.add
                )
                aug = aug2

            # ---------- h = relu(W1 @ target + b1) ---------------------------
            u0 = sbc.tile([128, 1], FP, tag="u0")
            nc.vector.tensor_scalar_mul(u0[:], w1n0[:, 1:2], tb[:, 1:2])
            hp0 = sbc.tile([128, 1], FP, tag="hp0")
            nc.vector.scalar_tensor_tensor(
                hp0[:], w1n0[:, 0:1], tb[:, 0:1], u0[:], ALU.mult, ALU.add
            )
            h0 = sbc.tile([128, 1], FP, tag="h0")
            nc.scalar.activation(h0[:], hp0[:], AFT.Relu, bias=b1a[:], scale=1.0)
            u1 = sbc.tile([128, 1], FP, tag="u1")
            nc.vector.tensor_scalar_mul(u1[:], w1n1[:, 1:2], tb[:, 1:2])
            hp1 = sbc.tile([128, 1], FP, tag="hp1")
            nc.vector.scalar_tensor_tensor(
                hp1[:], w1n1[:, 0:1], tb[:, 0:1], u1[:], ALU.mult, ALU.add
            )
            h1 = sbc.tile([128, 1], FP, tag="h1")
            nc.scalar.activation(h1[:], hp1[:], AFT.Relu, bias=b1b[:], scale=1.0)

            # ---------- G^T = (Wm @ P)^T  (independent of GJ) ----------------
            gtcp = psgc.tile([H, 3], FP, tag="gc")
            nc.tensor.matmul(gtcp[:, 0:2], p_sb[:], wmT[:])

            # ---------- x0 = W2 @ h + b2; c = P^-1 x0 ------------------------
            x0p = psa.tile([H, 1], FP, tag="mm")
            nc.tensor.matmul(x0p[:], w2t0[:], h0[:], start=True, stop=False)
            nc.tensor.matmul(x0p[:], w2t1[:], h1[:], start=False, stop=True)
            x0s = sbc.tile([H, 1], FP, tag="x0s")
            nc.scalar.activation(x0s[:], x0p[:], AFT.Identity, bias=b2s[:], scale=1.0)
            nc.tensor.matmul(gtcp[:, 2:3], aug[:, H : 2 * H], x0s[:])
            gts = sbc.tile([H, 2], FR, tag="gts")
            nc.vector.tensor_scalar_mul(gts[:], gtcp[:, 0:2], gtcp[:, 2:3])

            # ---------- actions: [1, 512] f32r matmuls + tanh ----------------
            ats = sbc.tile([1, F], FR, tag="ats")
            NJ = N_STEPS // 512
            for j in range(NJ):
                for ch in range(2):
                    bmt = bm0 if ch == 0 else bm1
                    atp = psact.tile([1, 512], FP, tag="actT")
                    nc.tensor.matmul(
                        atp[:],
                        gts[:, ch : ch + 1],
                        vcr[:, 512 * j : 512 * (j + 1)],
                    )
                    nc.scalar.activation(
                        ats[:, ch * N_STEPS + 512 * j : ch * N_STEPS + 512 * (j + 1)],
                        atp[:],
                        AFT.Tanh,
                        bias=bmt[:],
                        scale=1.0,
                    )

            # ---------- B[p, 2t+ch] = 15000 * ats[ch, t] on 128 partitions ---
            # Bsb = 15000*tanh for the fused vector path; B150 = Bsb/150 for
            # the gpsimd (tensor_add) + scalar (Copy x150) two-engine path.
            Bsb = sbc.tile([128, F], FP, tag="B")
            B3 = Bsb[:].rearrange("p (t m) -> p t m", m=2)
            B150 = sbc.tile([128, F], FP, tag="B150")
            B1503 = B150[:].rearrange("p (t m) -> p t m", m=2)
            for ch in range(2):
                for j in range(NJ):
                    bp = psB.tile([128, 512], FP, tag="B")
                    nc.tensor.matmul(
                        bp[:],
                        ones_r[:],
                        ats[:, ch * N_STEPS + 512 * j : ch * N_STEPS + 512 * (j + 1)],
                    )
                    nc.scalar.activation(
                        B3[:, 512 * j : 512 * (j + 1), ch : ch + 1],
                        bp[:, :, None],
                        AFT.Copy,
                        scale=15000.0,
                    )
                    nc.scalar.activation(
                        B1503[:, 512 * j : 512 * (j + 1), ch : ch + 1],
                        bp[:, :, None],
                        AFT.Copy,
                        scale=100.0,
                    )

            # ---------- main: out = 150*eps + B; engines and queues split ----
            for i, (t, r) in enumerate(eps_tiles):
                if i % 2 == 0:
                    nc.vector.scalar_tensor_tensor(
                        t[:], t[:], 150.0, Bsb[:], ALU.mult, ALU.add
                    )
                else:
                    # out = 150*(eps + B/150): add on gpsimd, scale on scalar
                    nc.gpsimd.tensor_add(t[:], t[:], B150[:])
                    nc.scalar.activation(t[:], t[:], AFT.Copy, scale=150.0)
                q = nc.scalar if i % 2 == 0 else nc.sync
                q.dma_start(out_d.ap()[r : r + 128, :], t[:])

    nc.compile()
    return nc


def get_nc():
    if "nc" not in _NC_CACHE:
        _NC_CACHE["nc"] = build_nc()
    return _NC_CACHE["nc"]


def kernel(**inputs):
    nc = get_nc()
    eps = np.ascontiguousarray(
        np.asarray(inputs["eps"], dtype=np.float32).reshape(N_ARMS, F)
    )
    small = {
        k: np.ascontiguousarray(np.asarray(inputs[k], dtype=np.float32))
        for k in ["target", "D", "P", "W1", "b1", "W2", "b2", "Wm", "bm"]
    }
    in_maps = [
        {**small, "eps": eps[i * ARMS_PER_CORE : (i + 1) * ARMS_PER_CORE]}
        for i in range(N_CORES)
    ]
    res = run_bass_kernel_spmd(nc, in_maps, core_ids=list(range(N_CORES)))
    out = np.concatenate([res.results[i]["out"] for i in range(N_CORES)], axis=0)
    return out.reshape(N_ARMS, 2, N_STEPS)


# revision 20
# speedup vs baseline: 1.0425x; 1.0425x over previous
"""Trainium2 Bass kernel for the arm-sampling rollout problem.

Math: the reference's 2048-step scan x <- x - (A@x)*dt with
A = P diag(exp(D)) P^-1 has the closed form
    hidden[k] = P diag(lam_i^k) P^-1 x0,   lam_i = 1 - dt*exp(D_i)
so actions^T[ch, k] = tanh(sum_i G[ch,i] * c_i * lam_i^k + bm[ch]) with
G = Wm @ P and c = P^-1 x0. c is obtained on-device: unpivoted
Gauss-Jordan on [P^T | I] (same pivot sequence as P; well-conditioned
for this problem family) gives Q = P^-T, then c = matmul(lhsT=Q, rhs=x0).
The output is the memory-bound broadcast
    out[arm, j] = 150*eps[arm, j] + 15000*act_flat[j]
over a [5000, 4096] array, 625 arms per core across 8 cores.

Perf notes (v2, from HW perfetto traces):
- A dma_start whose SBUF side has exactly 128 partitions is split across
  all 16 SDMA engines (1 descriptor per 16KB partition line, ~330GB/s);
  a 113-partition transfer collapses onto ONE engine (~27GB/s). So every
  bulk tile here is exactly 128 rows; the last tile overlaps the
  previous one by 15 rows (identical bytes are double-written).
- fp32 PE matmuls cost 4 cycles/row; float32r with moving dim >= 256
  costs 1. The broadcast matmuls (ones (x) actions -> B) and the action
  matmuls run in f32r (~19-bit mantissa, plenty for the 2e-2 gate).
- lam^k is built by 11 multiplicative doublings instead of
  iota+cast+ln+exp (saves ~10us of gpsimd/scalar critical path).
- The elementwise main loop is split vector/gpsimd, outputs alternate
  scalar/sync HWDGE queues so input and output streams stay spread.
"""

import numpy as np

import concourse.bass as bass
import concourse.bacc as bacc
import concourse.mybir as mybir
import concourse.tile as tile
from concourse.bass_utils import run_bass_kernel_spmd

N_ARMS = 5000
N_STEPS = 2048
H = 10
F = 2 * N_STEPS  # 4096 flattened per-arm elements
N_CORES = 8
ARMS_PER_CORE = N_ARMS // N_CORES  # 625
# All tiles exactly 128 rows (non-128-partition DMAs collapse onto one SDMA
# engine). 625 isn't divisible by 128, so two windows overlap by 15 rows;
# the overlapping pair is processed first/fourth so the framework's
# write-after-write ordering of their output DMAs never actually stalls.
OFFS = [ARMS_PER_CORE - 128, 128, 256, 384, 0]
FP = mybir.dt.float32
FR = mybir.dt.float32r

_NC_CACHE: dict = {}


def build_nc():
    AFT = mybir.ActivationFunctionType
    ALU = mybir.AluOpType

    nc = bacc.Bacc(
        "TRN2",
        target_bir_lowering=False,
        debug=False,
        enable_asserts=True,
        num_devices=N_CORES,
    )

    eps_d = nc.dram_tensor("eps", [ARMS_PER_CORE, F], FP, kind="ExternalInput")
    tgt_d = nc.dram_tensor("target", [2], FP, kind="ExternalInput")
    D_d = nc.dram_tensor("D", [H], FP, kind="ExternalInput")
    P_d = nc.dram_tensor("P", [H, H], FP, kind="ExternalInput")
    W1_d = nc.dram_tensor("W1", [256, 2], FP, kind="ExternalInput")
    b1_d = nc.dram_tensor("b1", [256], FP, kind="ExternalInput")
    W2_d = nc.dram_tensor("W2", [H, 256], FP, kind="ExternalInput")
    b2_d = nc.dram_tensor("b2", [H], FP, kind="ExternalInput")
    Wm_d = nc.dram_tensor("Wm", [2, H], FP, kind="ExternalInput")
    bm_d = nc.dram_tensor("bm", [2], FP, kind="ExternalInput")
    out_d = nc.dram_tensor("out", [ARMS_PER_CORE, F], FP, kind="ExternalOutput")

    with tile.TileContext(nc) as tc:
        with (
            tc.tile_pool(name="sbc", bufs=1) as sbc,
            tc.tile_pool(name="sbgj", bufs=2) as sbgj,
            tc.tile_pool(name="sbeps", bufs=1) as sbeps,
            tc.tile_pool(name="psa", bufs=2, space=bass.MemorySpace.PSUM) as psa,
            tc.tile_pool(name="psbc", bufs=1, space=bass.MemorySpace.PSUM) as psbc,
            tc.tile_pool(name="psgc", bufs=1, space=bass.MemorySpace.PSUM) as psgc,
            tc.tile_pool(name="psact", bufs=2, space=bass.MemorySpace.PSUM) as psact,
            tc.tile_pool(name="psB", bufs=2, space=bass.MemorySpace.PSUM) as psB,
        ):
            # ---------- bulk in: 5 x [128, F] transfers on sync --------------
            eps_tiles = []
            for i, r in enumerate(OFFS):
                t = sbeps.tile([128, F], FP, tag="eps" + str(i))
                nc.sync.dma_start(t[:], eps_d.ap()[r : r + 128, :])
                eps_tiles.append((t, r))

            # ---------- small loads: plain 2D on scalar HWDGE ----------------
            ds = sbc.tile([H, 1], FP, tag="ds")
            nc.scalar.dma_start(ds[:], D_d.ap()[:, None])
            # exp(D) immediately after the ds dispatch so lam/vc start early
            es = sbc.tile([H, 1], FP, tag="es")
            nc.scalar.activation(es[:], ds[:], mybir.ActivationFunctionType.Exp)
            tgtr = sbc.tile([1, 2], FP, tag="tgtr")
            nc.scalar.dma_start(tgtr[:], tgt_d.ap()[None, :])
            b1n = sbc.tile([1, 256], FP, tag="b1n")
            nc.scalar.dma_start(b1n[:], b1_d.ap()[None, :])
            w2n = sbc.tile([H, 256], FP, tag="w2n")
            nc.scalar.dma_start(w2n[:], W2_d.ap())
            p_sb = sbc.tile([H, H], FP, tag="p_sb")
            nc.scalar.dma_start(p_sb[:], P_d.ap())
            b2s = sbc.tile([H, 1], FP, tag="b2s")
            nc.scalar.dma_start(b2s[:], b2_d.ap()[:, None])

            # ---------- transposing/odd loads on gpsimd SWDGE ----------------
            pT = sbc.tile([H, H], FP, tag="pT")
            nc.gpsimd.dma_start(pT[:], P_d.ap().rearrange("m k -> k m"))
            idi = sbc.tile([H, H], mybir.dt.int32, tag="idi")
            nc.gpsimd.iota(idi[:], pattern=[[-1, H]], base=0, channel_multiplier=1)
            wmT = sbc.tile([H, 2], FP, tag="wmT")
            nc.gpsimd.dma_start(wmT[:], Wm_d.ap().rearrange("m k -> k m"))
            w1n0 = sbc.tile([128, 2], FP, tag="w1n0")
            nc.gpsimd.dma_start(w1n0[:], W1_d.ap()[0:128, :])
            w1n1 = sbc.tile([128, 2], FP, tag="w1n1")
            nc.gpsimd.dma_start(w1n1[:], W1_d.ap()[128:256, :])
            bm0 = sbc.tile([1, 1], FP, tag="bm0")
            nc.gpsimd.dma_start(bm0[:], bm_d.ap()[0:1][:, None])
            bm1 = sbc.tile([1, 1], FP, tag="bm1")
            nc.gpsimd.dma_start(bm1[:], bm_d.ap()[1:2][:, None])

            # ---------- vector-early consts ----------------------------------
            ones = sbc.tile([1, 128], FP, tag="ones")
            nc.vector.memset(ones[:], 1.0)
            ones_r = sbc.tile([1, 128], FR, tag="ones_r")
            nc.vector.tensor_copy(ones_r[:], ones[:])
            # idm[p, j] = 1 if p == j
            idm = sbc.tile([H, H], FP, tag="idm")
            nc.vector.tensor_scalar(idm[:], idi[:], 0, None, ALU.is_equal)
            # oht[:, 10k:10k+10] = matrix with row k all-ones (lhsT that
            # broadcasts row k of the GJ tableau to every partition).
            oht = sbc.tile([H, H * H], FP, tag="oht")
            oht3 = oht[:].rearrange("p (k r) -> p k r", r=H)
            for r in range(H):
                nc.vector.tensor_copy(oht3[:, :, r : r + 1], idm[:, :, None])

            # ---------- lam = 1 - 0.01*exp(D); vc[i,k] = lam_i^k -------------
            # runs on gpsimd (tensor_tensor ops only) to keep the vector
            # engine free for the Gauss-Jordan chain
            lam = sbc.tile([H, 1], FP, tag="lam")
            nc.vector.tensor_scalar(lam[:], es[:], -0.01, 1.0, ALU.mult, ALU.add)
            # multiplicative doubling: vc[:, w:2w] = vc[:, 0:w] * lam^w
            vc = sbc.tile([H, N_STEPS], FP, tag="vc")
            nc.vector.memset(vc[:, 0:1], 1.0)
            sqt = sbc.tile([H, 12], FP, tag="sqt")
            nc.gpsimd.tensor_copy(sqt[:, 0:1], lam[:])
            w, j = 1, 0
            while w < N_STEPS:
                nc.gpsimd.tensor_mul(
                    vc[:, w : 2 * w],
                    vc[:, 0:w],
                    sqt[:, j : j + 1].broadcast_to([H, w]),
                )
                if 2 * w < N_STEPS:
                    nc.gpsimd.tensor_mul(
                        sqt[:, j + 1 : j + 2], sqt[:, j : j + 1], sqt[:, j : j + 1]
                    )
                w, j = 2 * w, j + 1
            # f32r copy for the PE (consumers of f32r matmuls must be rounded)
            vcr = sbc.tile([H, N_STEPS], FR, tag="vcr")
            nc.scalar.activation(vcr[:], vc[:], AFT.Copy)

            # ---------- x0-path stationary transposes (PE, before GJ) --------
            tbp = psa.tile([128, 2], FP, tag="mm")
            nc.tensor.matmul(tbp[:], ones[:], tgtr[:])
            tb = sbc.tile([128, 2], FP, tag="tb")
            nc.vector.tensor_copy(tb[:], tbp[:])
            b1p0 = psa.tile([128, 1], FP, tag="mm")
            nc.tensor.matmul(
                b1p0[:], b1n[0:1, 0:128], ones[0:1, 0:1], is_transpose=True
            )
            b1a = sbc.tile([128, 1], FP, tag="b1a")
            nc.vector.tensor_copy(b1a[:], b1p0[:])
            b1p1 = psa.tile([128, 1], FP, tag="mm")
            nc.tensor.matmul(
                b1p1[:], b1n[0:1, 128:256], ones[0:1, 0:1], is_transpose=True
            )
            b1b = sbc.tile([128, 1], FP, tag="b1b")
            nc.vector.tensor_copy(b1b[:], b1p1[:])
            w2tp0 = psa.tile([128, H], FP, tag="mm")
            nc.tensor.matmul(w2tp0[:], w2n[:, 0:128], idm[:], is_transpose=True)
            w2t0 = sbc.tile([128, H], FP, tag="w2t0")
            nc.vector.tensor_copy(w2t0[:], w2tp0[:])
            w2tp1 = psa.tile([128, H], FP, tag="mm")
            nc.tensor.matmul(w2tp1[:], w2n[:, 128:256], idm[:], is_transpose=True)
            w2t1 = sbc.tile([128, H], FP, tag="w2t1")
            nc.vector.tensor_copy(w2t1[:], w2tp1[:])

            # ---------- Gauss-Jordan on [P^T | I] -> Q = P^-T ----------------
            aug = sbgj.tile([H, 2 * H], FP, tag="aug")
            nc.vector.tensor_copy(aug[:, 0:H], pT[:])
            nc.vector.tensor_copy(aug[:, H : 2 * H], idm[:])
            for k in range(H):
                # fn on gpsimd: runs concurrently with the PE broadcast
                fn = sbgj.tile([H, 1], FP, tag="fn")
                nc.gpsimd.tensor_sub(fn[:], idm[:, k : k + 1], aug[:, k : k + 1])
                bc = psbc.tile([H, 2 * H], FP, tag="bc")
                nc.tensor.matmul(bc[:], oht[:, H * k : H * k + H], aug[:])
                piv = sbgj.tile([H, 1], FP, tag="piv")
                nc.vector.reciprocal(piv[:], bc[:, k : k + 1])
                fn2 = sbgj.tile([H, 1], FP, tag="fn2")
                nc.vector.tensor_mul(fn2[:], fn[:], piv[:])
                aug2 = sbgj.tile([H, 2 * H], FP, tag="aug")
                nc.vector.scalar_tensor_tensor(
                    aug2[:], bc[:], fn2[:], aug[:], ALU.mult, ALU.add
                )
                aug = aug2

            # ---------- h = relu(W1 @ target + b1) on gpsimd/scalar ----------
            u0 = sbc.tile([128, 1], FP, tag="u0")
            nc.gpsimd.tensor_mul(u0[:], w1n0[:, 1:2], tb[:, 1:2])
            hp0 = sbc.tile([128, 1], FP, tag="hp0")
            nc.gpsimd.tensor_mul(hp0[:], w1n0[:, 0:1], tb[:, 0:1])
            hs0 = sbc.tile([128, 1], FP, tag="hs0")
            nc.gpsimd.tensor_add(hs0[:], hp0[:], u0[:])
            h0 = sbc.tile([128, 1], FP, tag="h0")
            nc.scalar.activation(h0[:], hs0[:], AFT.Relu, bias=b1a[:], scale=1.0)
            u1 = sbc.tile([128, 1], FP, tag="u1")
            nc.gpsimd.tensor_mul(u1[:], w1n1[:, 1:2], tb[:, 1:2])
            hp1 = sbc.tile([128, 1], FP, tag="hp1")
            nc.gpsimd.tensor_mul(hp1[:], w1n1[:, 0:1], tb[:, 0:1])
            hs1 = sbc.tile([128, 1], FP, tag="hs1")
            nc.gpsimd.tensor_add(hs1[:], hp1[:], u1[:])
            h1 = sbc.tile([128, 1], FP, tag="h1")
            nc.scalar.activation(h1[:], hs1[:], AFT.Relu, bias=b1b[:], scale=1.0)

            # ---------- G^T = (Wm @ P)^T  (independent of GJ) ----------------
            gtcp = psgc.tile([H, 3], FP, tag="gc")
            nc.tensor.matmul(gtcp[:, 0:2], p_sb[:], wmT[:])

            # ---------- x0 = W2 @ h + b2; c = P^-1 x0 ------------------------
            x0p = psa.tile([H, 1], FP, tag="mm")
            nc.tensor.matmul(x0p[:], w2t0[:], h0[:], start=True, stop=False)
            nc.tensor.matmul(x0p[:], w2t1[:], h1[:], start=False, stop=True)
            x0s = sbc.tile([H, 1], FP, tag="x0s")
            nc.scalar.activation(x0s[:], x0p[:], AFT.Identity, bias=b2s[:], scale=1.0)
            nc.tensor.matmul(gtcp[:, 2:3], aug[:, H : 2 * H], x0s[:])
            gts = sbc.tile([H, 2], FR, tag="gts")
            nc.vector.tensor_scalar_mul(gts[:], gtcp[:, 0:2], gtcp[:, 2:3])

            # ---------- actions: [1, 512] f32r matmuls + tanh ----------------
            ats = sbc.tile([1, F], FR, tag="ats")
            NJ = N_STEPS // 512
            for j in range(NJ):
                for ch in range(2):
                    bmt = bm0 if ch == 0 else bm1
                    atp = psact.tile([1, 512], FP, tag="actT")
                    nc.tensor.matmul(
                        atp[:],
                        gts[:, ch : ch + 1],
                        vcr[:, 512 * j : 512 * (j + 1)],
                    )
                    nc.scalar.activation(
                        ats[:, ch * N_STEPS + 512 * j : ch * N_STEPS + 512 * (j + 1)],
                        atp[:],
                        AFT.Tanh,
                        bias=bmt[:],
                        scale=1.0,
                    )

            # ---------- B[p, 2t+ch] = 15000 * ats[ch, t] on 128 partitions ---
            # PSUM->SBUF copies on the vector engine (idle here) so the
            # scalar engine only runs the tanh chain.
            Bsb = sbc.tile([128, F], FP, tag="B")
            B3 = Bsb[:].rearrange("p (t m) -> p t m", m=2)
            for ch in range(2):
                for j in range(NJ):
                    bp = psB.tile([128, 512], FP, tag="B")
                    nc.tensor.matmul(
                        bp[:],
                        ones_r[:],
                        ats[:, ch * N_STEPS + 512 * j : ch * N_STEPS + 512 * (j + 1)],
                    )
                    nc.vector.tensor_scalar_mul(
                        B3[:, 512 * j : 512 * (j + 1), ch : ch + 1],
                        bp[:, :, None],
                        15000.0,
                    )

            # ---------- main: out = 150*eps + B, all on vector ---------------
            # (concurrent Pool+DVE elementwise ops contend on SBUF ports and
            # each drop ~3x in rate, so a single fast engine wins)
            for i, (t, r) in enumerate(eps_tiles):
                nc.vector.scalar_tensor_tensor(
                    t[:], t[:], 150.0, Bsb[:], ALU.mult, ALU.add
                )
                q = nc.scalar if i % 2 == 0 else nc.sync
                q.dma_start(out_d.ap()[r : r + 128, :], t[:])

    nc.compile()
    return nc


def get_nc():
    if "nc" not in _NC_CACHE:
        _NC_CACHE["nc"] = build_nc()
    return _NC_CACHE["nc"]


def kernel(**inputs):
    nc = get_nc()
    eps = np.ascontiguousarray(
        np.asarray(inputs["eps"], dtype=np.float32).reshape(N_ARMS, F)
    )
    small = {
        k: np.ascontiguousarray(np.asarray(inputs[k], dtype=np.float32))
        for k in ["target", "D", "P", "W1", "b1", "W2", "b2", "Wm", "bm"]
    }
    in_maps = [
        {**small, "eps": eps[i * ARMS_PER_CORE : (i + 1) * ARMS_PER_CORE]}
        for i in range(N_CORES)
    ]
    res = run_bass_kernel_spmd(nc, in_maps, core_ids=list(range(N_CORES)))
    out = np.concatenate([res.results[i]["out"] for i in range(N_CORES)], axis=0)
    return out.reshape(N_ARMS, 2, N_STEPS)
